# revision 16
# baseline (speedup 1.0000x reference)
"""Trainium2 Bass kernel for nn_IntegerCifar10Net (quantized VGG-ish CNN).

Data parallel over 8 NeuronCores, B=64 images/core.

Design (v2, DoubleRow):
- Activations are stored as fp8e4m3 integers a = v+8 in [8, 15] (v in 0..7 =
  7*quantized value); padding rings hold 8.0 (v=0). The +8 offset makes all
  8 levels live on the unit-spaced fp8 grid; the per-channel psum offset
  C = 8*sum(w) is subtracted exactly (integer f32) before scaling.
- All convs after layer 1 run as fp8 DoubleRow matmuls (0.5 cyc/row): two
  K<=128 contraction units per matmul, paired via strided overlapping APs
  of the same activation tile (moving) and adjacent packed columns (weights).
- ci=64 layers (L2, L3) use a row-parity activation layout
  [128 = 64ch x (row%2)] so a single 128-partition window covers two dy taps;
  9 taps fit in 3 DR matmuls per 512-wide output group.
- ci=128 layers (L4, L5) pair taps directly: 5 DR matmuls (one zero-padded).
- ci=256 layer (L6) pairs the two 128-channel input groups per tap: 9 DR.
- Layer 1 keeps the exact bf16 hi/mid/lo im2col (K=81) so results match the
  fp32 reference bitwise.
- Quantize pipeline per group: [pool-x (DVE), pool-y (GPSIMD)] on raw psum,
  S1 = exact integer -C subtract (ScalarE Identity), S2 = s*u + bt
  (tensor_scalar, ptr scalars), S3 = +MAGIC / min MAGIC+7, S4 = max MAGIC /
  -(MAGIC-8) -> fp8: the round+clamp happens entirely in exact fp32 integer
  arithmetic, reproducing round-half-even bit-exactly.
"""

import sys
import numpy as np

sys.path.insert(0, "/opt/trn_rl_repo")

import ml_dtypes

N_CORES = 8
B = 64  # images per core
MAGIC = 12582912.0  # 1.5 * 2^23

# wall (fp8 weight blob) column offsets
OFF2, LEN2 = 0, 2 * 6 * 128
OFF3, LEN3 = 1536, 2 * 6 * 128
OFF4, LEN4 = 3072, 10 * 128
OFF5, LEN5 = 4352, 10 * 256
OFF6, LEN6 = 6912, 9 * 2 * 256
OFFF1, LENF1 = 11520, 32 * 512
OFFF2, LENF2 = 27904, 4 * 16
WT = 27968
NSLOT = 13


# ----------------------------------------------------------------------------
# Host-side packing
# ----------------------------------------------------------------------------

def _qint(w):
    w = np.asarray(w, np.float32)
    return np.round(np.clip(w, -1.0, 1.0) * np.float32(7.0)).astype(np.float64)


def _im2col_bf16x3(x):
    """x [Bc,3,32,32] f32 -> [81, 32, 32, Bc] bf16 (hi/mid/lo x 27 taps)."""
    Bc = x.shape[0]
    xp = np.zeros((Bc, 3, 34, 34), np.float32)
    xp[:, :, 1:33, 1:33] = x
    planes = np.empty((27, 32, 32, Bc), np.float32)
    k = 0
    for ci in range(3):
        for dy in range(3):
            for dx in range(3):
                planes[k] = np.transpose(xp[:, ci, dy:dy + 32, dx:dx + 32],
                                         (1, 2, 0))
                k += 1
    hi = planes.astype(ml_dtypes.bfloat16)
    r1 = planes - hi.astype(np.float32)
    mid = r1.astype(ml_dtypes.bfloat16)
    lo = (r1 - mid.astype(np.float32)).astype(ml_dtypes.bfloat16)
    return np.ascontiguousarray(np.concatenate([hi, mid, lo], axis=0))


def _pack_c64(wq):
    """wq [co, 64, 3, 3] -> [128, 2(pe), 6, co] (U0 U1 U2 V0 V1 V2)."""
    co = wq.shape[0]
    w = np.transpose(wq, (1, 2, 3, 0))  # [ci, dy, dx, co]
    out = np.zeros((128, 2, 6, co), np.float32)
    for dx in range(3):
        out[0:64, 0, dx] = w[:, 0, dx]
        out[64:128, 0, dx] = w[:, 1, dx]
        out[0:64, 0, 3 + dx] = w[:, 2, dx]
        out[0:64, 1, dx] = w[:, 1, dx]
        out[64:128, 1, dx] = w[:, 2, dx]
        out[64:128, 1, 3 + dx] = w[:, 0, dx]
    return out


def _pack_c64_pair(wq):
    """L2: [128, 2(rr), 6, 128] — row-in-pair rr selects psum half via
    zero-padded output columns ([w|0] for even rows, [0|w] for odd)."""
    base = _pack_c64(wq)  # [128, 2(pe), 6, 64]
    out = np.zeros((128, 2, 6, 128), np.float32)
    out[:, 0, :, 0:64] = base[:, 0]
    out[:, 1, :, 64:128] = base[:, 1]
    return out.reshape(128, 2 * 6 * 128)


def _pack_c128(wq):
    """wq [co, 128, 3, 3] -> [128, 10, co] taps flat + zero unit."""
    co = wq.shape[0]
    w = np.transpose(wq, (1, 2, 3, 0))
    out = np.zeros((128, 10, co), np.float32)
    for dy in range(3):
        for dx in range(3):
            out[:, dy * 3 + dx] = w[:, dy, dx]
    return out.reshape(128, 10 * co)


def _pack_c256(wq):
    """wq [co, 256, 3, 3] -> [128, 9, 2(cig), co]."""
    co = wq.shape[0]
    w = np.transpose(wq, (1, 2, 3, 0))  # [256, 3, 3, co]
    out = np.zeros((128, 9, 2, co), np.float32)
    for t in range(9):
        dy, dx = divmod(t, 3)
        out[:, t, 0] = w[0:128, dy, dx]
        out[:, t, 1] = w[128:256, dy, dx]
    return out.reshape(128, 9 * 2 * co)


def host_pack(inputs):
    f8 = ml_dtypes.float8_e4m3
    w_common = {}

    # L1 weights: [64,3,3,3] -> lhsT [27,64] tripled to [81,64] bf16
    t = np.transpose(_qint(inputs["w1"]).astype(np.float32),
                     (1, 2, 3, 0)).reshape(27, 64)
    w_common["w1sb"] = np.ascontiguousarray(
        np.concatenate([t, t, t], axis=0).astype(ml_dtypes.bfloat16))

    wall = np.zeros((128, WT), np.float32)
    wq = {i: _qint(inputs[f"w{i}"]) for i in range(2, 7)}
    wall[:, OFF2:OFF2 + LEN2] = _pack_c64_pair(wq[2].astype(np.float32))
    wall[:, OFF3:OFF3 + LEN3] = _pack_c64(
        wq[3].astype(np.float32)).reshape(128, LEN3)
    wall[:, OFF4:OFF4 + LEN4] = _pack_c128(wq[4].astype(np.float32))
    wall[:, OFF5:OFF5 + LEN5] = _pack_c128(wq[5].astype(np.float32))
    wall[:, OFF6:OFF6 + LEN6] = _pack_c256(wq[6].astype(np.float32))
    # FC1 [512, 4096], input index = (cig*128+p)*16 + px
    wf1 = _qint(inputs["wf1"]).astype(np.float32)  # [512, 4096]
    wf1k = wf1.reshape(512, 256, 16)  # [co, c, px]
    blk = np.zeros((128, 32, 512), np.float32)
    for cig in range(2):
        for px in range(16):
            blk[:, cig * 16 + px, :] = wf1k[:, cig * 128:(cig + 1) * 128,
                                            px].T
    wall[:, OFFF1:OFFF1 + LENF1] = blk.reshape(128, LENF1)
    wf2 = _qint(inputs["wf2"]).astype(np.float32)  # [10, 512]
    blk2 = np.zeros((128, 4, 16), np.float32)
    for kt in range(4):
        blk2[:, kt, 0:10] = wf2[:, kt * 128:(kt + 1) * 128].T
    wall[:, OFFF2:OFFF2 + LENF2] = blk2.reshape(128, LENF2)
    w_common["wall"] = np.ascontiguousarray(wall.astype(f8))

    # scale/bias slots: [128, NSLOT, 4] = [-C, s, bt, 1.0]
    sb = np.zeros((128, NSLOT, 4), np.float64)
    sb[:, :, 3] = 1.0

    def dup(v):
        return np.concatenate([v, v])

    g1 = np.asarray(inputs["g1"], np.float64)
    b1 = np.asarray(inputs["b1"], np.float64)
    sb[:, 0, 1] = dup(g1)
    sb[:, 0, 2] = dup(7.0 * b1)
    for i, slot in ((2, 1), (3, 2), (4, 3)):
        g = np.asarray(inputs[f"g{i}"], np.float64)
        b = np.asarray(inputs[f"b{i}"], np.float64)
        C = 8.0 * wq[i].sum(axis=(1, 2, 3))
        s = 7.0 * g / 49.0
        if i == 2:
            sb[:, slot, 0] = dup(-C)
            sb[:, slot, 1] = dup(s)
            sb[:, slot, 2] = dup(7.0 * b)
        else:
            sb[:, slot, 0] = -C
            sb[:, slot, 1] = s
            sb[:, slot, 2] = 7.0 * b
    for i, slot0 in ((5, 4), (6, 6)):
        g = np.asarray(inputs[f"g{i}"], np.float64)
        b = np.asarray(inputs[f"b{i}"], np.float64)
        C = 8.0 * wq[i].sum(axis=(1, 2, 3))
        s = 7.0 * g / 49.0
        for ct in range(2):
            sl = slice(ct * 128, (ct + 1) * 128)
            sb[:, slot0 + ct, 0] = -C[sl]
            sb[:, slot0 + ct, 1] = s[sl]
            sb[:, slot0 + ct, 2] = 7.0 * b[sl]
    gf1 = np.asarray(inputs["gf1"], np.float64)
    bf1 = np.asarray(inputs["bf1"], np.float64)
    Cf1 = 8.0 * _qint(inputs["wf1"]).sum(axis=1)
    sf1 = 7.0 * gf1 / 49.0
    for ct in range(4):
        sl = slice(ct * 128, (ct + 1) * 128)
        sb[:, 8 + ct, 0] = -Cf1[sl]
        sb[:, 8 + ct, 1] = sf1[sl]
        sb[:, 8 + ct, 2] = 7.0 * bf1[sl]
    gf2 = np.asarray(inputs["gf2"], np.float64)
    bf2 = np.asarray(inputs["bf2"], np.float64)
    Cf2 = 8.0 * _qint(inputs["wf2"]).sum(axis=1)
    sb[0:10, 12, 0] = -Cf2
    sb[0:10, 12, 1] = 7.0 * gf2 / 49.0
    sb[0:10, 12, 2] = 7.0 * bf2
    w_common["sball"] = np.ascontiguousarray(sb.astype(np.float32))

    x = np.asarray(inputs["x"], np.float32)
    maps = []
    for c in range(N_CORES):
        m = dict(w_common)
        m["xcol"] = _im2col_bf16x3(x[c * B:(c + 1) * B])
        maps.append(m)
    return maps


# ----------------------------------------------------------------------------
# Bass program
# ----------------------------------------------------------------------------

def build_nc():
    import concourse.bacc as bacc
    import concourse.mybir as mybir
    import concourse.tile as tile
    from concourse.ap import AP

    dt = mybir.dt
    AF = mybir.ActivationFunctionType
    OP = mybir.AluOpType
    FP8 = dt.float8e4
    PM = mybir.MatmulPerfMode

    nc = bacc.Bacc("TRN2", target_bir_lowering=False, debug=False)

    xcold = nc.dram_tensor("xcol", [81, 32, 32, B], dt.bfloat16,
                           kind="ExternalInput")
    w1d = nc.dram_tensor("w1sb", [81, 64], dt.bfloat16, kind="ExternalInput")
    walld = nc.dram_tensor("wall", [128, WT], FP8, kind="ExternalInput")
    sballd = nc.dram_tensor("sball", [128, NSLOT, 4], dt.float32,
                            kind="ExternalInput")
    outd = nc.dram_tensor("out", [B, 10], dt.float32, kind="ExternalOutput")

    def pair_ap(sl, stride):
        """Insert a [stride, 2] pair dim after the partition dim of slice."""
        ap = list(sl.ap)
        return AP(sl.tensor, sl.offset, [ap[0], [stride, 2]] + ap[1:])

    with tile.TileContext(nc) as tc:
        ENG = {}

        def eng(key):
            return {"v": nc.vector, "p": nc.gpsimd, "a": nc.scalar}[ENG[key]]

        class _CopyWrap:
            def __init__(self, e, is_a):
                self.e, self.is_a = e, is_a

            def copy(self, out, in_):
                if self.is_a:
                    self.e.copy(out, in_)
                else:
                    self.e.tensor_copy(out, in_)

        def engc(key):
            return _CopyWrap(eng(key), ENG[key] == "a")

        # ---------------- persistent weights -----------------------------
        wp_cm = tc.tile_pool(name="weights", bufs=1)
        wp = wp_cm.__enter__()
        w1 = wp.tile([81, 64], dt.bfloat16, tag="w1")
        nc.sync.dma_start(w1[:], w1d[:])
        wall = wp.tile([128, WT], FP8, tag="wall")
        nc.sync.dma_start(wall[:], walld[:])
        sball = wp.tile([128, NSLOT, 4], dt.float32, tag="sball")
        nc.sync.dma_start(sball[:], sballd[:])

        wl2 = wall[:, OFF2:OFF2 + LEN2].rearrange(
            "p (pe u c) -> p pe u c", pe=2, u=6)
        wl3 = wall[:, OFF3:OFF3 + LEN3].rearrange(
            "p (pe u c) -> p pe u c", pe=2, u=6)
        wl4 = wall[:, OFF4:OFF4 + LEN4].rearrange("p (u c) -> p u c", u=10)
        wl5 = wall[:, OFF5:OFF5 + LEN5].rearrange("p (u c) -> p u c", u=10)
        wl6 = wall[:, OFF6:OFF6 + LEN6].rearrange(
            "p (t h c) -> p t h c", t=9, h=2)
        wf1t = wall[:, OFFF1:OFFF1 + LENF1].rearrange(
            "p (u c) -> p u c", u=32)
        wf2t = wall[:, OFFF2:OFFF2 + LENF2].rearrange(
            "p (u c) -> p u c", u=4)

        def sN(slot, col, np_=128, base=0):
            return sball[base:base + np_, slot, col:col + 1]

        # S2: z = s*z + bt (ptr scalars); on ACT via activation(Identity)
        def s2(key, z_ap, slot, np_=128, base=0):
            if ENG[key] == "a":
                nc.scalar.activation(z_ap, z_ap, AF.Identity,
                                     bias=sN(slot, 2, np_, base),
                                     scale=sN(slot, 1, np_, base))
            else:
                eng(key).tensor_scalar(z_ap, z_ap, sN(slot, 1, np_, base),
                                       sN(slot, 2, np_, base),
                                       OP.mult, OP.add)

        # merged pooled-layer quantize: TS1 (add -C, mult s), TS2 (add bt,
        # add MAGIC), TS3 (min MAGIC+7, max MAGIC), TS4 (sub MAGIC-8) -> fp8
        def quant_ts(pfx, z_ap, slot, np_, base, dst_ap, z4=None):
            eng(pfx + ".q1").tensor_scalar(z_ap, z_ap, sN(slot, 0, np_, base),
                                           sN(slot, 1, np_, base),
                                           OP.add, OP.mult)
            eng(pfx + ".q2").tensor_scalar(z_ap, z_ap, sN(slot, 2, np_, base),
                                           MAGIC, OP.add, OP.add)
            eng(pfx + ".q3").tensor_scalar(z_ap, z_ap, MAGIC + 7.0, MAGIC,
                                           OP.min, OP.max)
            eng(pfx + ".q4").tensor_scalar(dst_ap,
                                           z4 if z4 is not None else z_ap,
                                           MAGIC - 8.0, None, OP.subtract)

        # quantize tail: S3 = (+MAGIC, min MAGIC+7); S4 = (max MAGIC,
        # -(MAGIC-8)) -> fp8
        def s3s4(key3, key4, z_ap, dst_ap, z4=None):
            eng(key3).tensor_scalar(z_ap, z_ap, MAGIC, MAGIC + 7.0,
                                    OP.add, OP.min)
            eng(key4).tensor_scalar(dst_ap, z4 if z4 is not None else z_ap,
                                    MAGIC, MAGIC - 8.0, OP.max, OP.subtract)

        # ---------------- activation tiles --------------------------------
        a_cms = [tc.tile_pool(name=f"A{i}", bufs=1) for i in range(2, 9)]
        a_pools = [cm.__enter__() for cm in a_cms]
        A2 = a_pools[0].tile([128, 17, 34, B], FP8, tag="A2")
        A3 = a_pools[1].tile([128, 9, 18, B], FP8, tag="A3")
        A4 = a_pools[2].tile([128, 18, 18, B], FP8, tag="A4")
        A5 = a_pools[3].tile([128, 10, 10, B], FP8, tag="A5")
        A6 = a_pools[4].tile([128, 2, 10, 10, B], FP8, tag="A6")
        A7 = a_pools[5].tile([128, 2, 4, 4, B], FP8, tag="A7")
        A8 = a_pools[6].tile([128, 4, B], FP8, tag="A8")

        # border pad rings = 8.0 (v=0)
        nc.gpsimd.memset(A2[0:64, 0, :, :], 8.0)
        nc.gpsimd.memset(A2[64:128, 16, :, :], 8.0)
        nc.gpsimd.memset(A2[:, :, 0, :], 8.0)
        nc.gpsimd.memset(A2[:, :, 33, :], 8.0)
        nc.gpsimd.memset(A3[0:64, 0, :, :], 8.0)
        nc.gpsimd.memset(A3[64:128, 8, :, :], 8.0)
        nc.gpsimd.memset(A3[:, :, 0, :], 8.0)
        nc.gpsimd.memset(A3[:, :, 17, :], 8.0)
        nc.gpsimd.memset(A4[:, 0, :, :], 8.0)
        nc.gpsimd.memset(A4[:, 17, :, :], 8.0)
        nc.gpsimd.memset(A4[:, 1:17, 0, :], 8.0)
        nc.gpsimd.memset(A4[:, 1:17, 17, :], 8.0)
        nc.gpsimd.memset(A5[:, 0, :, :], 8.0)
        nc.gpsimd.memset(A5[:, 9, :, :], 8.0)
        nc.gpsimd.memset(A5[:, 1:9, 0, :], 8.0)
        nc.gpsimd.memset(A5[:, 1:9, 9, :], 8.0)
        nc.gpsimd.memset(A6[:, :, 0, :, :], 8.0)
        nc.gpsimd.memset(A6[:, :, 9, :, :], 8.0)
        nc.gpsimd.memset(A6[:, :, 1:9, 0, :], 8.0)
        nc.gpsimd.memset(A6[:, :, 1:9, 9, :], 8.0)

        # ---------------- Layer 1: exact bf16 K=81 im2col -----------------
        ENG.update({"1.3": "v", "1.4": "p"})
        with (tc.tile_pool(name="l1x", bufs=2) as pxc,
              tc.tile_pool(name="l1ps", bufs=3, space="PSUM") as pps,
              tc.tile_pool(name="l1z", bufs=3) as pz):
            xcs = []
            for c in range(8):
                xc = pxc.tile([81, 4, 32, B], dt.bfloat16, tag="xc")
                nc.sync.dma_start(xc[:], xcold[:, 4 * c:4 * c + 4, :, :])
                xcs.append(xc)
            groups = [(0,)] + [(y, y + 1) for y in range(1, 31, 2)] + [(31,)]
            for rows in groups:
                idx = (rows[0] + 1) // 2
                np_ = 64 * len(rows)
                base = 64 if len(rows) == 1 and rows[0] == 0 else 0
                for half in range(2):
                    ps = pps.tile([np_, 2, 512], dt.float32, tag="ps")
                    for y in rows:
                        ph = 0 if (y % 2 == 1) else 1
                        if len(rows) == 1:
                            orow = ps
                        else:
                            orow = ps[64 * ph:64 * ph + 64]
                        for gg in range(2):
                            g = 2 * half + gg
                            rhs = xcs[y // 4][:, y % 4, 8 * g:8 * g + 8, :]
                            nc.tensor.matmul(orow[:, gg, :], w1[:], rhs,
                                             start=True, stop=True)
                    z = pz.tile([np_, 2, 512], dt.float32, tag="z")
                    nc.scalar.activation(z[:], ps[:], AF.Relu,
                                         bias=sN(0, 2, np_, base),
                                         scale=sN(0, 1, np_, base))
                    dst = A2[base:base + np_, idx,
                             1 + 16 * half:17 + 16 * half, :]
                    s3s4("1.3", "1.4",
                         z[:].rearrange("p a b -> p (a b)"),
                         dst.rearrange("p x b -> p (x b)"))

        # ---------------- ci=64 parity conv helper ------------------------
        def emit_c64_group(Ain, wlp, idxstride, y, g, out_ap,
                           start0=True, stopN=True):
            pe = y % 2
            if pe == 0:
                iu, iv = y // 2, y // 2 + 1
            else:
                iu, iv = (y + 1) // 2, (y - 1) // 2
            x0 = 8 * g
            rhs1 = pair_ap(Ain[:, iu, x0:x0 + 8, :], B)
            rhs2 = pair_ap(Ain[:, iu, x0 + 2:x0 + 10, :],
                           (iv - iu) * idxstride - 2 * B)
            rhs3 = pair_ap(Ain[:, iv, x0 + 1:x0 + 9, :], B)
            nc.tensor.matmul(out_ap, wlp[:, 0:2, :], rhs1, start=start0,
                             stop=False, perf_mode=PM.DoubleRow)
            nc.tensor.matmul(out_ap, wlp[:, 2:4, :], rhs2, start=False,
                             stop=False, perf_mode=PM.DoubleRow)
            nc.tensor.matmul(out_ap, wlp[:, 4:6, :], rhs3, start=False,
                             stop=stopN, perf_mode=PM.DoubleRow)

        # ---------------- Layer 2 (64->64, 32x32, pool) -------------------
        ENG.update({"2.c": "a", "2.x": "v", "2.y": "v", "2.r": "p",
                    "2.q1": "p", "2.q2": "p", "2.q3": "v", "2.q4": "p"})
        IDX2 = 34 * B
        with (tc.tile_pool(name="l2ps", bufs=3, space="PSUM") as pps,
              tc.tile_pool(name="l2t", bufs=3) as pt,
              tc.tile_pool(name="l2st", bufs=2) as pst):
            st = None
            for yo in range(16):
                t1 = pt.tile([128, 2, 512], dt.float32, tag="t1")
                for h in range(2):
                    ps = pps.tile([128, 2, 512], dt.float32, tag="ps")
                    for rr in range(2):
                        y = 2 * yo + rr
                        for gg in range(2):
                            emit_c64_group(A2, wl2[:, rr], IDX2, y,
                                           2 * h + gg, ps[:, gg, :],
                                           start0=(rr == 0),
                                           stopN=(rr == 1))
                    pse = ps[:].rearrange("p g (x two b) -> p g x two b",
                                          two=2, b=B)
                    t1v = t1[:, h, :].rearrange("p (g x b) -> p g x b",
                                                g=2, b=B)
                    engc("2.c").copy(t1v, pse[:, :, :, 0, :])
                    eng("2.x").tensor_tensor(t1v, t1v, pse[:, :, :, 1, :],
                                             OP.max)
                if yo == 0:
                    st = pst.tile([64, 1024], dt.float32, tag="st0")
                    sty = st[:]
                elif yo % 2 == 1:
                    st = pst.tile([128, 1024], dt.float32, tag="st")
                    sty = st[0:64, :]
                else:
                    sty = st[64:128, :]
                tr = pt.tile([64, 1024], dt.float32, tag="tr")
                engc("2.r").copy(tr[:],
                                 t1[64:128, :, :].rearrange("p a b -> p (a b)"))
                eng("2.y").tensor_tensor(
                    sty, t1[0:64, :, :].rearrange("p a b -> p (a b)"),
                    tr[:], OP.max)
                if yo == 0:
                    fin = (64, 64, 0, st[:])
                elif yo % 2 == 0:
                    fin = (128, 0, yo // 2, st[:])
                elif yo == 15:
                    fin = (64, 0, 8, st[0:64, :])
                else:
                    fin = None
                if fin is not None:
                    np_, base, i, src = fin
                    dst = A3[base:base + np_, i, 1:17, :]
                    quant_ts("2", src, 1, np_, base,
                             dst.rearrange("p x b -> p (x b)"))

        # ---------------- Layer 3 (64->128, 16x16) ------------------------
        ENG.update({"3.1": "a", "3.2": "v", "3.3": "v", "3.4": "p"})
        IDX3 = 18 * B
        with (tc.tile_pool(name="l3ps", bufs=3, space="PSUM") as pps,
              tc.tile_pool(name="l3z", bufs=3) as pz):
            for y in range(16):
                ps = pps.tile([128, 2, 512], dt.float32, tag="ps")
                for g in range(2):
                    emit_c64_group(A3, wl3[:, y % 2], IDX3, y, g,
                                   ps[:, g, :])
                z = pz.tile([128, 2, 512], dt.float32, tag="z")
                nc.scalar.activation(z[:], ps[:], AF.Identity,
                                     bias=sN(2, 0), scale=1.0)
                zf = z[:].rearrange("p a b -> p (a b)")
                s2("3.2", zf, 2)
                s3s4("3.3", "3.4", zf,
                     A4[:, 1 + y, 1:17, :].rearrange("p x b -> p (x b)"))

        # ---------------- ci=128 tap-pair conv helper ---------------------
        # tap pairs (0,1),(2,3),(4,5),(6,7),(8,zero)
        def emit_c128_group(Ain, wl, rowstride, y, x0, out_ap, ct):
            strides = [B, rowstride - 2 * B, B, B, -2 * B]
            for k in range(5):
                t = 2 * k
                dy, dx = divmod(t, 3) if t < 9 else (2, 2)
                base = Ain[:, y + dy, x0 + dx:x0 + dx + 8, :]
                rhs = pair_ap(base, strides[k])
                if ct is None:
                    w = wl[:, t:t + 2, :]
                else:
                    w = wl[:, t:t + 2, 128 * ct:128 * ct + 128]
                nc.tensor.matmul(out_ap, w, rhs, start=(k == 0),
                                 stop=(k == 4), perf_mode=PM.DoubleRow)

        # ---------------- Layer 4 (128->128, 16x16, pool) -----------------
        ENG.update({"4.c": "a", "4.x": "v", "4.y": "v", "4.q1": "p",
                    "4.q2": "p", "4.q3": "v", "4.q4": "p"})
        with (tc.tile_pool(name="l4ps", bufs=3, space="PSUM") as pps,
              tc.tile_pool(name="l4t", bufs=3) as pt,
              tc.tile_pool(name="l4st", bufs=2) as pst):
            st = None
            for yo in range(8):
                t1 = pt.tile([128, 2, 512], dt.float32, tag="t1")
                for rr in range(2):
                    y = 2 * yo + rr
                    ps = pps.tile([128, 2, 512], dt.float32, tag="ps")
                    for g in range(2):
                        emit_c128_group(A4, wl4, 18 * B, y, 8 * g,
                                        ps[:, g, :], None)
                    pse = ps[:].rearrange("p g (x two b) -> p g x two b",
                                          two=2, b=B)
                    t1v = t1[:, rr, :].rearrange("p (g x b) -> p g x b",
                                                 g=2, b=B)
                    engc("4.c").copy(t1v, pse[:, :, :, 0, :])
                    eng("4.x").tensor_tensor(t1v, t1v, pse[:, :, :, 1, :],
                                             OP.max)
                if yo % 2 == 0:
                    st = pst.tile([128, 2, 512], dt.float32, tag="st")
                eng("4.y").tensor_tensor(st[:, yo % 2, :], t1[:, 0, :],
                                         t1[:, 1, :], OP.max)
                if yo % 2 == 1:
                    zf = st[:].rearrange("p a b -> p (a b)")
                    k = yo // 2
                    dst = A5[:, 1 + 2 * k:3 + 2 * k, 1:9, :]
                    quant_ts("4", zf, 3, 128, 0, dst,
                             st[:].rearrange("p a (x b) -> p a x b", b=B))

        # ---------------- Layer 5 (128->256, 8x8) -------------------------
        ENG.update({"5.1": "a", "5.2": "p", "5.3": "v", "5.4": "p"})
        with (tc.tile_pool(name="l5ps", bufs=3, space="PSUM") as pps,
              tc.tile_pool(name="l5z", bufs=3) as pz):
            for yp in range(4):
                for ct in range(2):
                    ps = pps.tile([128, 2, 512], dt.float32, tag="ps")
                    for rr in range(2):
                        y = 2 * yp + rr
                        emit_c128_group(A5, wl5, 10 * B, y, 0,
                                        ps[:, rr, :], ct)
                    z = pz.tile([128, 2, 512], dt.float32, tag="z")
                    nc.scalar.activation(z[:], ps[:], AF.Identity,
                                         bias=sN(4 + ct, 0), scale=1.0)
                    zf = z[:].rearrange("p a b -> p (a b)")
                    s2("5.2", zf, 4 + ct)
                    dst = A6[:, ct, 1 + 2 * yp:3 + 2 * yp, 1:9, :]
                    s3s4("5.3", "5.4", zf, dst,
                         z[:].rearrange("p a (x b) -> p a x b", b=B))

        # ---------------- Layer 6 (256->256, 8x8, pool) -------------------
        ENG.update({"6.c": "a", "6.x": "v", "6.y": "v", "6.q1": "p",
                    "6.q2": "p", "6.q3": "v", "6.q4": "p"})
        CIG = 10 * 10 * B
        with (tc.tile_pool(name="l6ps", bufs=3, space="PSUM") as pps,
              tc.tile_pool(name="l6t", bufs=3) as pt,
              tc.tile_pool(name="l6st", bufs=2) as pst):
            for ct in range(2):
                st = pst.tile([128, 4, 256], dt.float32, tag="st")
                for yo in range(4):
                    ps = pps.tile([128, 2, 512], dt.float32, tag="ps")
                    for rr in range(2):
                        y = 2 * yo + rr
                        for t in range(9):
                            dy, dx = divmod(t, 3)
                            rhs = pair_ap(
                                A6[:, 0, y + dy, dx:dx + 8, :], CIG)
                            nc.tensor.matmul(
                                ps[:, rr, :],
                                wl6[:, t, :, 128 * ct:128 * ct + 128], rhs,
                                start=(t == 0), stop=(t == 8),
                                perf_mode=PM.DoubleRow)
                    t1 = pt.tile([128, 2, 256], dt.float32, tag="t1")
                    pse = ps[:].rearrange("p r (x two b) -> p r x two b",
                                          two=2, b=B)
                    t1v = t1[:].rearrange("p r (x b) -> p r x b", b=B)
                    engc("6.c").copy(t1v, pse[:, :, :, 0, :])
                    eng("6.x").tensor_tensor(t1v, t1v, pse[:, :, :, 1, :],
                                             OP.max)
                    eng("6.y").tensor_tensor(st[:, yo, :], t1[:, 0, :],
                                             t1[:, 1, :], OP.max)
                zf = st[:].rearrange("p a b -> p (a b)")
                quant_ts("6", zf, 6 + ct, 128, 0,
                         A7[:, ct, :, :, :].rearrange("p y x b -> p (y x b)"))

        # ---------------- FC1 (4096 -> 512) -------------------------------
        ENG.update({"f.2": "v", "f.3": "v", "f.4": "p"})
        with (tc.tile_pool(name="f1ps", bufs=1, space="PSUM") as pps,
              tc.tile_pool(name="f1z", bufs=1) as pz):
            ps = pps.tile([128, 4, B], dt.float32, tag="ps")
            for ct in range(4):
                for j in range(16):
                    u = 2 * j
                    cig, px = divmod(u, 16)
                    base = A7[:, cig, px // 4, px % 4, :]
                    rhs = pair_ap(base, B)
                    nc.tensor.matmul(ps[:, ct, :],
                                     wf1t[:, u:u + 2,
                                          128 * ct:128 * ct + 128],
                                     rhs, start=(j == 0), stop=(j == 15),
                                     perf_mode=PM.DoubleRow)
            z = pz.tile([128, 4, B], dt.float32, tag="z")
            for ct in range(4):
                nc.scalar.activation(z[:, ct, :], ps[:, ct, :], AF.Identity,
                                     bias=sN(8 + ct, 0), scale=1.0)
                s2("f.2", z[:, ct, :], 8 + ct)
            zf = z[:].rearrange("p a b -> p (a b)")
            s3s4("f.3", "f.4", zf,
                 A8[:].rearrange("p a b -> p (a b)"))

        # ---------------- FC2 (512 -> 10) + final transform ---------------
        with (tc.tile_pool(name="f2ps", bufs=1, space="PSUM") as pps,
              tc.tile_pool(name="f2t", bufs=1) as pt):
            ps = pps.tile([10, B], dt.float32, tag="ps")
            for k in range(2):
                kt = 2 * k
                rhs = A8[:, kt:kt + 2, :]
                nc.tensor.matmul(ps[:], wf2t[:, kt:kt + 2, 0:10], rhs,
                                 start=(k == 0), stop=(k == 1),
                                 perf_mode=PM.DoubleRow)
            z = pt.tile([10, B], dt.float32, tag="z")
            nc.scalar.activation(z[:], ps[:], AF.Identity,
                                 bias=sN(12, 0, 10), scale=1.0)
            nc.vector.tensor_scalar(z[:], z[:], sN(12, 1, 10),
                                    sN(12, 2, 10), OP.mult, OP.add)
            nc.vector.tensor_scalar(z[:], z[:], MAGIC, MAGIC + 7.0,
                                    OP.add, OP.min)
            nc.vector.tensor_scalar(z[:], z[:], MAGIC - 7.0, MAGIC,
                                    OP.max, OP.subtract)
            fin = pt.tile([10, B], dt.float32, tag="fin")
            nc.vector.tensor_scalar(fin[:], z[:], 1.0 / 7.0, None, OP.mult)
            nc.sync.dma_start(outd[:].rearrange("b c -> c b"), fin[:])

        for cm in reversed(a_cms):
            cm.__exit__(None, None, None)
        wp_cm.__exit__(None, None, None)



    nc.compile()
    return nc


# ----------------------------------------------------------------------------
# Entry point
# ----------------------------------------------------------------------------

_NC_CACHE = {}


def kernel(**inputs):
    from concourse.bass_utils import run_bass_kernel_spmd
    if "nc" not in _NC_CACHE:
        _NC_CACHE["nc"] = build_nc()
    nc = _NC_CACHE["nc"]
    in_maps = host_pack(inputs)
    res = run_bass_kernel_spmd(nc, in_maps, list(range(N_CORES)))
    _NC_CACHE["last_results"] = res
    outs = [res.results[c]["out"] for c in range(N_CORES)]
    return np.concatenate(outs, axis=0).astype(np.float32)
